# revision 1
# baseline (speedup 1.0000x reference)
"""Trainium2 Bass kernel for CapsNet dynamic routing (nn_CapsRoutingLayer).

Reference computation (see problem):
    x_hat[b,i,o,d] = sum_k W[i,o,d,k] * x[b,i,k]
    b_logits = 0
    for it in 0..2:
        c = softmax_o(b_logits); s[b,o,d] = sum_i c[b,i,o] x_hat[b,i,o,d]
        v = squash(s)   # global Frobenius norm over whole s tensor
        if it < 2: b_logits += sum_d x_hat[b,i,o,d] v[b,o,d]
    return v  # (128, 32, 32)

Sharding: input capsules i (1152) split across 8 cores (144 each). W shard
(18.9MB fp32) stays resident in SBUF; x_hat is regenerated on the fly each
routing iteration via PE matmuls (4x row-tiled, K=32) and consumed from PSUM
by DVE ops, so the 604MB x_hat tensor never exists in full. The per-iteration
partial s (and the squash norm) are combined across cores with an on-device
AllReduce of the tiny [128,1024] s tensor.

Host-side prep: per-core shards of W and x are pre-transposed with numpy into
the SBUF layouts the kernel wants:
    ws[(g,k), j, (o,d)] = W[i0 + 4j+g, o, d, k]     (128, 36, 1024) fp32
    xs[(g,k), j, b]     = x[b, i0 + 4j+g, k]        (128, 36, 128)  fp32
"""

import numpy as np

from concourse import bacc, bass_isa, bass_utils, mybir, tile

N_CORES = 8
B = 128          # batch
NI = 1152        # input capsules
K = 32           # dim_input
NO = 32          # output capsules
D = 32           # dim_output
IC = NI // N_CORES   # input capsules per core = 144
NJ = IC // 4         # i-groups of 4 per core = 36
OD = NO * D          # 1024

F32 = mybir.dt.float32
ADD = mybir.AluOpType.add
MULT = mybir.AluOpType.mult
AXX = mybir.AxisListType.X
EXP = mybir.ActivationFunctionType.Exp

# s-accumulation strategy: PSUM_SACC=True accumulates c*x_hat in PSUM via
# identity-stationary PE matmuls (NGRP must be 3 to leave 2 banks free);
# False accumulates on DVE with tensor_add (NGRP=4, all 8 banks for x_hat).
PSUM_SACC = False
NGRP = 4
# Timing-ablation only: replace the cross-core AllReduce with a plain DMA
# (results become wrong; used to measure the collective's cost).
SKIP_COLLECTIVE = False


def _kernel_body(nc, tc, xs, ws, id_in, vout, repeats=1):
    with tc.tile_pool(name="main", bufs=1) as main, \
         tc.tile_pool(name="psum", bufs=1, space="PSUM") as psum, \
         tc.tile_pool(name="dram", bufs=1, space="DRAM") as dram:

        W_t = main.tile([128, NJ, OD], F32)
        x_t = main.tile([128, NJ, 128], F32)

        if PSUM_SACC:
            ident = main.tile([128, 128], F32)
            nc.sync.dma_start(ident[:], id_in[:])
            cx = main.tile([B, OD], F32)     # weighted x_hat staging for PE
            s_acc = None
        else:
            ident = cx = None
            s_acc = main.tile([B, OD], F32)
        v_cur = main.tile([B, OD], F32)
        s_full = main.tile([B, OD], F32)
        tmp2 = main.tile([B, 2 * OD], F32)   # scratch for 2-capsule slabs
        a4 = main.tile([B, NGRP * NO], F32)  # logits, NGRP capsules x 32 o
        a_old = main.tile([B, NGRP * NO], F32)
        e4 = main.tile([B, NGRP * NO], F32)
        c4 = main.tile([B, NGRP * NO], F32)
        z4 = main.tile([B, NGRP], F32)
        rz4 = main.tile([B, NGRP], F32)
        ones128 = main.tile([128, 1], F32)
        ones1 = main.tile([1, 128], F32)
        nrm1 = main.tile([1, 1], F32)
        s_sq = main.tile([B, 1], F32)
        g_sc = main.tile([B, 1], F32)
        t1 = main.tile([B, 1], F32)
        t2 = main.tile([B, 1], F32)
        t3 = main.tile([B, 1], F32)
        nc.vector.memset(ones128[:], 1.0)
        nc.vector.memset(ones1[:], 1.0)

        ar_in = dram.tile([B, OD], F32)
        ar_out = dram.tile([B, OD], F32)
        a_dram = dram.tile([B, IC // NGRP, NGRP * NO], F32)

        pg = psum.tile([B, NGRP * OD], F32)  # x_hat tiles
        s_ps = psum.tile([B, OD], F32) if PSUM_SACC else None

        def allreduce_s(src):
            nc.sync.dma_start(ar_in[:], src)
            if SKIP_COLLECTIVE:
                nc.sync.dma_start(ar_out[:], ar_in[:])
            else:
                nc.gpsimd.collective_compute(
                    "AllReduce", ADD,
                    replica_groups=[list(range(N_CORES))],
                    ins=[ar_in.opt()], outs=[ar_out.opt()],
                )
            nc.sync.dma_start(s_full[:], ar_out[:])

        def squash(accumulate=False):
            # g = sqrt(S)/(1+S), S = global sum of squares of s_full.
            # accumulate=False: v_cur = g*s_full.
            # accumulate=True:  v_cur += g*s_full (routing logits are linear
            # in v, so pass 2 can use v0+v1 as its agreement multiplier).
            nc.vector.tensor_mul(tmp2[:, 0:OD], s_full[:], s_full[:])
            # reduce over partitions via PE (ones stationary), then over free
            for h in range(2):
                nc.tensor.matmul(pg[0:1, 512 * h:512 * (h + 1)], ones128[:],
                                 tmp2[:, 512 * h:512 * (h + 1)],
                                 start=True, stop=True)
            nc.vector.tensor_reduce(out=nrm1[:], in_=pg[0:1, 0:OD],
                                    axis=AXX, op=ADD)
            # broadcast the scalar back to all 128 partitions via PE
            nc.tensor.matmul(pg[:, 0:1], ones1[:], nrm1[:],
                             start=True, stop=True)
            nc.vector.tensor_copy(s_sq[:], pg[:, 0:1])
            nc.scalar.sqrt(t1[:], s_sq[:])
            nc.vector.tensor_scalar_add(t2[:], s_sq[:], 1.0)
            nc.vector.reciprocal(t3[:], t2[:])
            nc.vector.tensor_mul(g_sc[:], t1[:], t3[:])
            if accumulate:
                nc.vector.tensor_scalar_mul(s_full[:], s_full[:], g_sc[:])
                nc.vector.tensor_add(v_cur[:], v_cur[:], s_full[:])
            else:
                nc.vector.tensor_scalar_mul(v_cur[:], s_full[:], g_sc[:])

        # ---- repeats > 1 is a timing aid: the whole computation re-runs
        # serially (same tiles, deps chain), so (T(R)-T(1))/(R-1) isolates
        # one full iteration including the W/x loads.
        for _rep in range(repeats):
            _run_once(nc, pg, s_ps, W_t, x_t, xs, ws, ident,
                      allreduce_s, squash, s_full, s_acc, v_cur,
                      tmp2, cx, a4, a_old, e4, c4, z4, rz4, a_dram)

        nc.sync.dma_start(vout[:], v_cur[:])


def _run_once(nc, pg, s_ps, W_t, x_t, xs, ws, ident, allreduce_s, squash,
              s_full, s_acc, v_cur, tmp2, cx, a4, a_old, e4, c4, z4, rz4,
              a_dram):
        nc.sync.dma_start(W_t[:], ws[:])
        nc.sync.dma_start(x_t[:], xs[:])

        # ---- pass 0: s0 = (1/32) sum_i x_hat[b,i,:,:], direct K=128 matmuls
        for h in range(2):
            for j in range(NJ):
                nc.tensor.matmul(
                    pg[:, 512 * h:512 * (h + 1)],
                    x_t[:, j, :], W_t[:, j, 512 * h:512 * (h + 1)],
                    start=(j == 0), stop=(j == NJ - 1))
        nc.vector.tensor_scalar_mul(tmp2[:, 0:OD], pg[:, 0:OD], 1.0 / NO)
        allreduce_s(tmp2[:, 0:OD])
        squash()

        # ---- passes 1, 2: groups of NGRP capsules; i -> (j=i//4, g=i%4)
        NG = IC // NGRP
        for r in (1, 2):
            if not PSUM_SACC:
                nc.vector.memset(s_acc[:], 0.0)
            for t in range(NG):
                # regenerate x_hat for NGRP capsules into PSUM (row-tiled)
                for slot in range(NGRP):
                    i = NGRP * t + slot
                    j, g = i // 4, i % 4
                    for h in range(2):
                        lo = slot * OD + 512 * h
                        nc.tensor.matmul(
                            pg[:, lo:lo + 512],
                            x_t[32 * g:32 * (g + 1), j, :],
                            W_t[32 * g:32 * (g + 1), j, 512 * h:512 * (h + 1)],
                            start=True, stop=True, tile_position=(32 * g, 0))
                # agreement logits: a4[b, (slot,o)] = sum_d x_hat * v_cur
                for half in range(NGRP // 2):
                    ph = pg[:, half * 2 * OD:(half + 1) * 2 * OD]
                    nc.vector.tensor_tensor(
                        out=tmp2[:].rearrange("b (i f) -> b i f", i=2),
                        in0=ph.rearrange("b (i f) -> b i f", i=2),
                        in1=v_cur[:].unsqueeze(1).broadcast_to([B, 2, OD]),
                        op=MULT)
                    nc.vector.tensor_reduce(
                        out=a4[:, half * 64:(half + 1) * 64]
                            .rearrange("b (i o) -> b i o", i=2),
                        in_=tmp2[:].rearrange("b (i o d) -> b i o d",
                                              i=2, o=NO),
                        axis=AXX, op=ADD)
                # c4 = softmax over o (logits are tiny; skip max-subtraction)
                nc.scalar.activation(e4[:], a4[:], EXP)
                nc.vector.tensor_reduce(
                    out=z4[:], in_=e4[:].rearrange("b (i o) -> b i o", i=NGRP),
                    axis=AXX, op=ADD)
                nc.vector.reciprocal(rz4[:], z4[:])
                nc.vector.tensor_tensor(
                    out=c4[:].rearrange("b (i o) -> b i o", i=NGRP),
                    in0=e4[:].rearrange("b (i o) -> b i o", i=NGRP),
                    in1=rz4[:].unsqueeze(2).broadcast_to([B, NGRP, NO]),
                    op=MULT)
                # s += c*x_hat: one double-width multiply per capsule PAIR
                # (fewer DVE ops -> fewer pipeline DRAINs), adds on GpSimd
                if PSUM_SACC:
                    for slot in range(NGRP):
                        nc.vector.tensor_tensor(
                            out=cx[:].rearrange("b (o d) -> b o d", o=NO),
                            in0=pg[:, slot * OD:(slot + 1) * OD]
                                .rearrange("b (o d) -> b o d", o=NO),
                            in1=c4[:, slot * NO:(slot + 1) * NO]
                                .unsqueeze(2).broadcast_to([B, NO, D]),
                            op=MULT)
                        first = (t == 0 and slot == 0)
                        last = (t == NG - 1 and slot == NGRP - 1)
                        for h in range(2):
                            nc.tensor.matmul(
                                s_ps[:, 512 * h:512 * (h + 1)], ident[:],
                                cx[:, 512 * h:512 * (h + 1)],
                                start=first, stop=last)
                else:
                    for half in range(NGRP // 2):
                        nc.vector.tensor_tensor(
                            out=tmp2[:].rearrange("b (i o d) -> b i o d",
                                                  i=2, o=NO),
                            in0=pg[:, half * 2 * OD:(half + 1) * 2 * OD]
                                .rearrange("b (i o d) -> b i o d", i=2, o=NO),
                            in1=c4[:, half * 64:(half + 1) * 64]
                                .rearrange("b (i o) -> b i o", i=2)
                                .unsqueeze(3).broadcast_to([B, 2, NO, D]),
                            op=MULT)
                        nc.gpsimd.tensor_add(s_acc[:], s_acc[:],
                                             tmp2[:, 0:OD])
                        nc.gpsimd.tensor_add(s_acc[:], s_acc[:],
                                             tmp2[:, OD:2 * OD])
            if PSUM_SACC:
                nc.vector.tensor_copy(tmp2[:, 0:OD], s_ps[:])
                allreduce_s(tmp2[:, 0:OD])
            else:
                allreduce_s(s_acc[:])
            # pass 1: v_cur <- v0 + v1 (joint agreement multiplier for
            # pass 2); pass 2: v_cur <- v2 (the output)
            squash(accumulate=(r == 1))


_NC_CACHE = {}


def _build(repeats=1):
    if repeats in _NC_CACHE:
        return _NC_CACHE[repeats]
    nc = bacc.Bacc("TRN2", target_bir_lowering=False, debug=False,
                   num_devices=N_CORES)
    xs = nc.dram_tensor("xs", [128, NJ, 128], F32, kind="ExternalInput").ap()
    ws = nc.dram_tensor("ws", [128, NJ, OD], F32, kind="ExternalInput").ap()
    id_in = nc.dram_tensor("ident", [128, 128], F32, kind="ExternalInput").ap()
    vout = nc.dram_tensor("v", [B, OD], F32, kind="ExternalOutput").ap()
    with tile.TileContext(nc) as tc:
        _kernel_body(nc, tc, xs, ws, id_in, vout, repeats=repeats)
    nc.compile()
    _NC_CACHE[repeats] = nc
    return nc


def _shard_inputs(x, W):
    in_maps = []
    for c in range(N_CORES):
        i0 = c * IC
        wc = W[i0:i0 + IC]                          # (144, 32, 32, 32) iodk
        ws = np.ascontiguousarray(
            wc.reshape(NJ, 4, NO, D, K).transpose(1, 4, 0, 2, 3)
              .reshape(128, NJ, OD)).astype(np.float32, copy=False)
        xc = x[:, i0:i0 + IC, :]                    # (128, 144, 32) bik
        xt = np.ascontiguousarray(
            xc.reshape(B, NJ, 4, K).transpose(2, 3, 1, 0)
              .reshape(128, NJ, 128)).astype(np.float32, copy=False)
        in_maps.append({"xs": xt, "ws": ws,
                        "ident": np.eye(128, dtype=np.float32)})
    return in_maps


def kernel(x, W, _trace=False):
    x = np.asarray(x, dtype=np.float32)
    W = np.asarray(W, dtype=np.float32)
    nc = _build()
    in_maps = _shard_inputs(x, W)
    res = bass_utils.run_bass_kernel_spmd(
        nc, in_maps, core_ids=list(range(N_CORES)), trace=_trace)
    out = res.results[0]["v"].reshape(B, NO, D).astype(np.float32, copy=False)
    if _trace:
        kernel.last_exec_time_ns = res.exec_time_ns
        kernel.last_results = res
    return out



# revision 4
# speedup vs baseline: 2.4870x; 2.4870x over previous
"""Trainium2 Bass kernel for CapsNet dynamic routing (nn_CapsRoutingLayer).

Reference computation:
    x_hat[b,i,o,d] = sum_k W[i,o,d,k] * x[b,i,k]
    b_logits = 0
    for it in 0..2:
        c = softmax_o(b_logits); s[b,o,d] = sum_i c[b,i,o] x_hat[b,i,o,d]
        v = squash(s)   # global Frobenius norm over whole s tensor
        if it < 2: b_logits += sum_d x_hat[b,i,o,d] v[b,o,d]
    return v  # (128, 32, 32)

Sharding: input capsules i (1152) split across 8 cores (144 each); the tiny
per-iteration s [128,1024] is AllReduced.

Dataflow (all data bf16 except PSUM accumulations, all layouts (d,o)-major):
  pass 0: s0 = (1/32) sum_i x_hat via K=128 PE matmuls (no per-capsule x_hat).
  passes 1,2 per capsule c: PE regenerates x_hat[b, (d,o)] into a 3-slot PSUM
  ring (K=32 matmuls, 4-way row-tiled); ScalarE drains it once to SBUF bf16
  (the only PSUM read per element). Per 8-capsule super-group the DVE then
  runs entirely on SBUF bf16 at 2x packing:
    TT-p:  p = x_hat * v-broadcast      (agreement product)
    tree:  5 halving adds over d -> logits a[b,(i,o)]
    softmax (exp on ScalarE), TT-q: q = x_hat * c-broadcast
  and the PE accumulates s = sum_i q with identity-stationary matmuls into a
  dedicated PSUM bank pair (fp32 accumulate for free). Emission is software-
  pipelined (gen k+1 before ident k) so the PE runs ahead and the next pass's
  generation hides the AllReduce+squash latency.

Host-side prep per core:
    ws[(g,k), j, (d,o)] = W[i0 + 4j+g, o, d, k]     (128, 36, 1024) bf16
    xs[(g,k), j, b]     = x[b, i0 + 4j+g, k]        (128, 36, 128)  bf16
Output v comes back [B, (d,o)] fp32 and is transposed on the host.
"""

import numpy as np

from concourse import bacc, bass_utils, mybir, tile
from concourse.bass_isa import ReduceOp

N_CORES = 8
B = 128          # batch
NI = 1152        # input capsules
K = 32           # dim_input
NO = 32          # output capsules
D = 32           # dim_output
IC = NI // N_CORES   # input capsules per core = 144
NJ = IC // 4         # i-groups of 4 per core = 36
OD = NO * D          # 1024

SG = 8           # capsules per DVE super-group
NSG = IC // SG   # 18
RING = 2 * SG    # x_hat bf16 ring depth (capsules)
NPS = 3          # PSUM x_hat ring depth (capsules)

F32 = mybir.dt.float32
BF16 = mybir.dt.bfloat16
ADD = mybir.AluOpType.add
MULT = mybir.AluOpType.mult
AXX = mybir.AxisListType.X
EXP = mybir.ActivationFunctionType.Exp
COPY = mybir.ActivationFunctionType.Copy
RADD = ReduceOp.add


def _kernel_body(nc, tc, xs_in, ws_in, id_in, vout, repeats=1):
    with tc.tile_pool(name="main", bufs=1) as main, \
         tc.tile_pool(name="psum", bufs=1, space="PSUM") as psum, \
         tc.tile_pool(name="dram", bufs=1, space="DRAM") as dram:

        ws_t = main.tile([128, NJ, OD], BF16)
        xs_t = main.tile([128, NJ, 128], BF16)
        ident = main.tile([128, 128], BF16)

        xh_bf = main.tile([B, RING * OD], BF16)   # drained x_hat ring
        p_t = main.tile([B, SG * OD], BF16)       # agreement product
        q_t = main.tile([B, 2, SG * OD], BF16)    # weighted product (2 bufs;
                                                  # tree temps alias into it)
        lgt = main.tile([B, SG * NO], BF16)       # logits
        e_t = main.tile([B, SG * NO], BF16)
        c_t = main.tile([B, SG * NO], BF16)
        z_t = main.tile([B, SG], F32)
        rz_t = main.tile([B, SG], F32)

        s_sb = main.tile([B, OD], F32)            # local s (pre-AllReduce)
        s_full = main.tile([B, OD], F32)          # global s (post-AllReduce)
        sq_t = main.tile([B, OD], F32)
        vm = main.tile([B, OD], BF16)             # squash output (cumulative)
        vfin = main.tile([B, OD], F32)
        t1 = main.tile([B, 1], F32)
        t2 = main.tile([B, 1], F32)
        t3 = main.tile([B, 1], F32)
        g_sc = main.tile([B, 1], F32)

        ar_in = dram.tile([B, OD], F32)
        ar_out = dram.tile([B, OD], F32)

        xh_ps = psum.tile([B, NPS * OD], F32)     # 6 banks
        s_ps = psum.tile([B, OD], F32)            # 2 banks

        nc.sync.dma_start(ident[:], id_in[:])
        for _rep in range(repeats):
            _run_once(nc, xs_in, ws_in, ws_t, xs_t, ident, xh_bf, p_t, q_t,
                      lgt, e_t, c_t, z_t, rz_t, s_sb, s_full, sq_t, vm, vfin,
                      t1, t2, t3, g_sc, ar_in, ar_out, xh_ps, s_ps, vout)


def _run_once(nc, xs_in, ws_in, ws_t, xs_t, ident, xh_bf, p_t, q_t, lgt, e_t,
              c_t, z_t, rz_t, s_sb, s_full, sq_t, vm, vfin, t1, t2, t3, g_sc,
              ar_in, ar_out, xh_ps, s_ps, vout):
    # ---- load inputs (W chunked so pass-0 matmuls start early)
    nc.sync.dma_start(xs_t[:], xs_in[:])
    WCH = 6
    for tch in range(NJ // WCH):
        nc.sync.dma_start(ws_t[:, WCH * tch:WCH * (tch + 1), :],
                          ws_in[:, WCH * tch:WCH * (tch + 1), :])

    def allreduce_s():
        nc.sync.dma_start(ar_in[:], s_sb[:])
        nc.gpsimd.collective_compute(
            "AllReduce", ADD,
            replica_groups=[list(range(N_CORES))],
            ins=[ar_in.opt()], outs=[ar_out.opt()],
        )
        nc.sync.dma_start(s_full[:], ar_out[:])

    def squash(accumulate, final=False):
        # g = sqrt(S)/(1+S), S = global sum of squares of s_full
        nc.vector.tensor_mul(sq_t[:], s_full[:], s_full[:])
        nc.vector.tensor_reduce(out=t1[:], in_=sq_t[:], axis=AXX, op=ADD)
        nc.gpsimd.partition_all_reduce(t1[:], t1[:], 128, RADD)
        nc.scalar.sqrt(t2[:], t1[:])
        nc.vector.tensor_scalar_add(t3[:], t1[:], 1.0)
        nc.vector.reciprocal(t3[:], t3[:])
        nc.vector.tensor_mul(g_sc[:], t2[:], t3[:])
        if final:
            nc.vector.tensor_scalar_mul(vfin[:], s_full[:], g_sc[:])
            nc.sync.dma_start(vout[:], vfin[:])
            return
        if accumulate:
            nc.vector.tensor_scalar_mul(sq_t[:], s_full[:], g_sc[:])
            nc.vector.tensor_add(vm[:], vm[:], sq_t[:])
        else:
            nc.vector.tensor_scalar_mul(vm[:], s_full[:], g_sc[:])

    # ---- pass 0: s0 = (1/32) sum_i x_hat, direct K=128 matmuls
    for j in range(NJ):
        for h in range(2):
            nc.tensor.matmul(
                s_ps[:, 512 * h:512 * (h + 1)],
                xs_t[:, j, :], ws_t[:, j, 512 * h:512 * (h + 1)],
                start=(j == 0), stop=(j == NJ - 1))
    nc.vector.tensor_scalar_mul(s_sb[:], s_ps[:], 1.0 / NO)
    allreduce_s()
    squash(accumulate=False)

    # ---- passes 1, 2: software-pipelined per 8-capsule super-group
    for r in (1, 2):
        def gen_drain(k):
            for cc in range(SG):
                c = SG * k + cc
                j, g = c // 4, c % 4
                slot = c % NPS
                for h in range(2):
                    lo = slot * OD + 512 * h
                    nc.tensor.matmul(
                        xh_ps[:, lo:lo + 512],
                        xs_t[32 * g:32 * (g + 1), j, :],
                        ws_t[32 * g:32 * (g + 1), j, 512 * h:512 * (h + 1)],
                        start=True, stop=True, tile_position=(32 * g, 0))
                rs = c % RING
                nc.scalar.activation(
                    xh_bf[:, OD * rs:OD * (rs + 1)],
                    xh_ps[:, OD * slot:OD * (slot + 1)], COPY)

        def dve_ident(k):
            pb = k % 2
            base = (SG * k) % RING
            xb = xh_bf[:, OD * base:OD * (base + SG)]
            xb4 = xb.rearrange("b (s d o) -> b s d o", s=SG, d=D)
            # agreement product p = x_hat * v (broadcast over capsules)
            nc.vector.tensor_tensor(
                out=p_t[:].rearrange("b (s d o) -> b s d o", s=SG, d=D),
                in0=xb4,
                in1=vm[:].rearrange("b (d o) -> b d o", d=D)
                    .unsqueeze(1).broadcast_to([B, SG, D, NO]),
                op=MULT)
            # tree-reduce over d: temps alias into this super-group's q buf
            qf = q_t[:, pb, :]
            offs = [0, SG * 16 * NO, SG * 24 * NO, SG * 28 * NO]
            src, sd = p_t[:], D
            for lv in range(4):
                w = sd // 2
                dst = qf[:, offs[lv]:offs[lv] + SG * w * NO]
                nc.vector.tensor_tensor(
                    out=dst.rearrange("b (s d o) -> b s d o", s=SG, d=w),
                    in0=src.rearrange("b (s d o) -> b s d o", s=SG, d=sd)
                        [:, :, 0:w, :],
                    in1=src.rearrange("b (s d o) -> b s d o", s=SG, d=sd)
                        [:, :, w:sd, :],
                    op=ADD)
                src, sd = dst, w
            nc.vector.tensor_tensor(
                out=lgt[:].rearrange("b (s o) -> b s o", s=SG)
                    .unsqueeze(2),
                in0=src.rearrange("b (s d o) -> b s d o", s=SG, d=2)
                    [:, :, 0:1, :],
                in1=src.rearrange("b (s d o) -> b s d o", s=SG, d=2)
                    [:, :, 1:2, :],
                op=ADD)
            # softmax over o (logits tiny; skip max-subtraction)
            nc.scalar.activation(e_t[:], lgt[:], EXP)
            nc.vector.tensor_reduce(
                out=z_t[:], in_=e_t[:].rearrange("b (s o) -> b s o", s=SG),
                axis=AXX, op=ADD)
            nc.vector.reciprocal(rz_t[:], z_t[:])
            nc.vector.tensor_tensor(
                out=c_t[:].rearrange("b (s o) -> b s o", s=SG),
                in0=e_t[:].rearrange("b (s o) -> b s o", s=SG),
                in1=rz_t[:].unsqueeze(2).broadcast_to([B, SG, NO]),
                op=MULT)
            # weighted product q = x_hat * c (broadcast over d: middle dim)
            nc.vector.tensor_tensor(
                out=qf.rearrange("b (s d o) -> b s d o", s=SG, d=D),
                in0=xb4,
                in1=c_t[:].rearrange("b (s o) -> b s o", s=SG)
                    .unsqueeze(2).broadcast_to([B, SG, D, NO]),
                op=MULT)
            # s += q via identity-stationary PE accumulation into s_ps
            for cc in range(SG):
                c = SG * k + cc
                for h in range(2):
                    nc.tensor.matmul(
                        s_ps[:, 512 * h:512 * (h + 1)], ident[:],
                        qf[:, OD * cc + 512 * h:OD * cc + 512 * (h + 1)],
                        start=(c == 0), stop=(c == IC - 1))

        for k in range(NSG + 1):
            if k < NSG:
                gen_drain(k)
            if k >= 1:
                dve_ident(k - 1)

        nc.vector.tensor_copy(s_sb[:], s_ps[:])
        allreduce_s()
        # r=1: vm <- v0+v1 (joint agreement multiplier for pass 2);
        # r=2: final output v2
        squash(accumulate=True, final=(r == 2))


_NC_CACHE = {}


def _build(repeats=1):
    if repeats in _NC_CACHE:
        return _NC_CACHE[repeats]
    nc = bacc.Bacc("TRN2", target_bir_lowering=False, debug=False,
                   num_devices=N_CORES)
    xs = nc.dram_tensor("xs", [128, NJ, 128], BF16, kind="ExternalInput").ap()
    ws = nc.dram_tensor("ws", [128, NJ, OD], BF16, kind="ExternalInput").ap()
    id_in = nc.dram_tensor("ident", [128, 128], BF16,
                           kind="ExternalInput").ap()
    vout = nc.dram_tensor("v", [B, OD], F32, kind="ExternalOutput").ap()
    with tile.TileContext(nc) as tc:
        _kernel_body(nc, tc, xs, ws, id_in, vout, repeats=repeats)
    nc.compile()
    _NC_CACHE[repeats] = nc
    return nc


def _shard_inputs(x, W):
    bf = mybir.dt.np(BF16)
    in_maps = []
    for c in range(N_CORES):
        i0 = c * IC
        wc = W[i0:i0 + IC]                          # (144, 32, 32, 32) iodk
        ws = np.ascontiguousarray(
            wc.reshape(NJ, 4, NO, D, K).transpose(1, 4, 0, 3, 2)
              .reshape(128, NJ, OD)).astype(bf)
        xc = x[:, i0:i0 + IC, :]                    # (128, 144, 32) bik
        xt = np.ascontiguousarray(
            xc.reshape(B, NJ, 4, K).transpose(2, 3, 1, 0)
              .reshape(128, NJ, 128)).astype(bf)
        in_maps.append({"xs": xt, "ws": ws,
                        "ident": np.eye(128, dtype=np.float32).astype(bf)})
    return in_maps


def kernel(x, W, _trace=False):
    x = np.asarray(x, dtype=np.float32)
    W = np.asarray(W, dtype=np.float32)
    nc = _build()
    in_maps = _shard_inputs(x, W)
    res = bass_utils.run_bass_kernel_spmd(
        nc, in_maps, core_ids=list(range(N_CORES)), trace=_trace)
    vdo = res.results[0]["v"].reshape(B, D, NO)     # (b, d, o) layout
    out = np.ascontiguousarray(vdo.transpose(0, 2, 1)).astype(np.float32)
    if _trace:
        kernel.last_exec_time_ns = res.exec_time_ns
        kernel.last_results = res
    return out
